# revision 1
# baseline (speedup 1.0000x reference)
"""Trainium2 kernel for nn_CCLoss (retrieval_knn, K=5 nearest-color loss).

Strategy (data-parallel over bs=8 across 8 cores, one sample per core):
  host: replicate the reference's grid_sample gather exactly (tiny), build
        per-sample matmul operands so that the PE computes
           negdist_shifted[l, p] = 2*sum_c pool[l,c]*img[c,p] - sum_c img[c,p]^2
                                 = -dist[l,p] + ||pool_l||^2   (row-const shift)
        Row-constant shifts don't change per-row top-K ranking.
  device (per core):
        K=4 float32r matmul -> PSUM [128, chunk] where partitions 0-63 carry
        (l, pixel half 0) and 64-127 carry (l, pixel half 1); ACT copies the
        PSUM chunks into two [128, 16384] SBUF buffers; DVE Max8 + MaxIndex
        produce the top-8 (value, index) per partition-row per buffer.
  host: merge the 32 candidates per (b, l) exactly (value desc, index asc),
        take top-5, then replicate the remainder of the reference loss.
"""

import os
import numpy as np

import concourse.bass as bass
import concourse.tile as tile
from concourse import bacc, mybir
from concourse.bass_utils import run_bass_kernel_spmd

BS, L, CH, IMG = 8, 64, 3, 256
NPIX = IMG * IMG            # 65536 pixels
HALF = NPIX // 2            # 32768 pixels per partition-half
CHUNK = 1024                # pixels per matmul/copy iteration per half
NITER = HALF // CHUNK       # 32
# per-piece sizes in chunks: small first piece (earlier DVE start) and small
# last piece (shorter serial tail); must sum to NITER.
PIECES = [2, 4, 4, 4, 4, 4, 4, 4, 2]
NPIECE = len(PIECES)
assert sum(PIECES) == NITER
FOLD_LEVELS = 4             # fold to w/2^FOLD_LEVELS slots per piece
SLOTPIX = 2 ** FOLD_LEVELS  # source pixels covered by one fold slot
K = 5

TRACE = False               # test.py sets this for profiling runs
LAST_RESULT = None          # test.py reads exec_time_ns / trace info here

_NC = None


def _emit_body(nc, tc, pools, lhsT, rhs_ext, idxs_ext, warm):
    rhs_pool, psum_pool, res_pool = pools
    # PE pipeline warm-up: a few matmuls on a zeroed tile ramp the tensor
    # engine to full clock while the first rhs DMAs are still in flight.
    wps = psum_pool.tile([128, CHUNK], mybir.dt.float32, tag="ps")
    for _ in range(3):
        nc.tensor.matmul(wps[:, 0:512], warm[:, 0:128],
                         warm[:], start=True, stop=True)
    chunk_base = 0
    for g, nchunks in enumerate(PIECES):
        # Per pair of chunks: ACT drains both chunks PSUM->SBUF as bf16,
        # DVE folds them at the 2x bf16 rate, then runs the rest of the
        # fold tree. Slot s of the folded piece covers piece pixels
        # s + (w/SLOTPIX)*t, t in [0, SLOTPIX); losers are recovered on
        # the host by exact re-scoring of all SLOTPIX pixels per slot.
        w = nchunks * CHUNK
        f1 = res_pool.tile([128, w // 2], mybir.dt.bfloat16, tag="f1")
        for h in range(nchunks // 2):
            ce = res_pool.tile([128, CHUNK], mybir.dt.bfloat16, tag="c0")
            co = res_pool.tile([128, CHUNK], mybir.dt.bfloat16, tag="c1")
            for par in range(2):
                i = chunk_base + 2 * h + par
                ra = rhs_pool.tile([10, CHUNK], mybir.dt.float32r, tag="rhs")
                dma_eng = nc.sync if par == 0 else nc.gpsimd
                dma_eng.dma_start(ra[:], rhs_ext[:, bass.ts(i, CHUNK)])

                ps = psum_pool.tile([128, CHUNK], mybir.dt.float32, tag="ps")
                for j in range(CHUNK // 512):
                    nc.tensor.matmul(
                        ps[:, bass.ts(j, 512)], lhsT[:],
                        ra[:, bass.ts(j, 512)], start=True, stop=True)
                nc.scalar.activation(
                    (ce if par == 0 else co)[:], ps[:],
                    mybir.ActivationFunctionType.Copy)
            nc.vector.tensor_max(f1[:, bass.ts(h, CHUNK)], ce[:], co[:])

        fk = f1
        wk = w // 2
        for _lev in range(FOLD_LEVELS - 1):
            nk = res_pool.tile([128, wk // 2], mybir.dt.bfloat16,
                               tag=f"f{_lev + 2}")
            nc.vector.tensor_max(nk[:], fk[:, :wk // 2], fk[:, wk // 2:])
            fk, wk = nk, wk // 2

        mx = res_pool.tile([128, 8], mybir.dt.bfloat16, tag="mx")
        nc.vector.max(mx[:], fk[:])
        mi = res_pool.tile([128, 8], mybir.dt.uint32, tag="mi")
        nc.vector.max_index(mi[:], mx[:], fk[:])
        nc.sync.dma_start(idxs_ext[:, bass.ts(g, 8)], mi[:])
        chunk_base += nchunks


def _build(loop_reps=None):
    nc = bacc.Bacc("TRN2", target_bir_lowering=False, debug=False)
    lhsT_ext = nc.declare_dram_parameter(
        "lhsT", [10, 128], mybir.dt.float32r, isOutput=False)
    rhs_ext = nc.declare_dram_parameter(
        "rhs", [10, HALF], mybir.dt.float32r, isOutput=False)
    idxs_ext = nc.declare_dram_parameter(
        "out_idx", [128, NPIECE * 8], mybir.dt.uint32, isOutput=True)

    with tile.TileContext(nc) as tc:
        with tc.tile_pool(name="consts", bufs=1) as consts, \
             tc.tile_pool(name="rhs", bufs=6) as rhs_pool, \
             tc.tile_pool(name="psum", bufs=4, space="PSUM") as psum_pool, \
             tc.tile_pool(name="res", bufs=3) as res_pool:

            lhsT = consts.tile([10, 128], mybir.dt.float32r)
            nc.gpsimd.dma_start(lhsT[:], lhsT_ext[:])
            warm = consts.tile([10, 512], mybir.dt.float32)
            nc.vector.memset(warm[:], 0.0)
            pools = (rhs_pool, psum_pool, res_pool)
            if loop_reps is None:
                _emit_body(nc, tc, pools, lhsT, rhs_ext, idxs_ext, warm)
            else:
                with tc.For_i(0, loop_reps, 1):
                    _emit_body(nc, tc, pools, lhsT, rhs_ext, idxs_ext, warm)
    nc.compile()
    return nc


def _pooled_host(predictions, ref_imgs):
    """Verbatim numpy replication of the reference grid_sample block."""
    pos = predictions[:, :, :2].astype(np.float32)
    pos_flat = pos.reshape(BS * L, 2)
    img_idx = np.arange(BS * L) % BS
    coord = pos_flat * np.float32(IMG) - np.float32(0.5)
    ix = np.rint(coord[:, 0]).astype(np.int32)
    iy = np.rint(coord[:, 1]).astype(np.int32)
    valid = (ix >= 0) & (ix < IMG) & (iy >= 0) & (iy < IMG)
    ixc = np.clip(ix, 0, IMG - 1)
    iyc = np.clip(iy, 0, IMG - 1)
    pooled_flat = (ref_imgs[img_idx, :, iyc, ixc]
                   * valid[:, None].astype(ref_imgs.dtype))
    pooled = pooled_flat.reshape(L, BS, CH).transpose(1, 0, 2)  # [bs, L, ch]
    return pos, pooled.astype(np.float32)


def _prepare_inputs(predictions, ref_imgs):
    """Build per-core matmul operands. The PE computes a true -dist:
         v[l,p] = 2*sum_c pool*img - sum_c img^2 - sum_c pool^2
    The -||pool||^2 row centers the per-row maxima near 0 so the bf16 fold
    tree keeps ~2^-9 *relative* resolution right where ranking happens.
    lhsT [10,128] block-diagonal: rows 0-4 -> partitions 0-63 (pixel half 0),
    rows 5-9 -> partitions 64-127 (half 1)."""
    pos, pooled = _pooled_host(predictions, ref_imgs)
    imgs_flat = ref_imgs.reshape(BS, CH, NPIX).astype(np.float32)
    s = (imgs_flat * imgs_flat).sum(axis=1, dtype=np.float32)   # [bs, NPIX]
    normsq = (pooled * pooled).sum(axis=-1, dtype=np.float32)   # [bs, L]

    coef = np.empty((BS, 5, L), dtype=np.float32)
    coef[:, :CH, :] = 2.0 * pooled.transpose(0, 2, 1)
    coef[:, CH, :] = -1.0           # multiplies the s row
    coef[:, CH + 1, :] = -normsq    # multiplies the ones row
    lhsT_np = np.zeros((BS, 10, 128), dtype=np.float32)
    lhsT_np[:, 0:5, 0:L] = coef
    lhsT_np[:, 5:10, L:128] = coef

    ones = np.ones((BS, 1, NPIX), dtype=np.float32)
    rhs_full = np.concatenate(
        [imgs_flat, s[:, None, :], ones], axis=1)               # [bs,5,NPIX]
    rhs_np = np.concatenate(
        [rhs_full[:, :, :HALF], rhs_full[:, :, HALF:]], axis=1)  # [bs,10,HALF]
    in_maps = [{"lhsT": np.ascontiguousarray(lhsT_np[b]),
                "rhs": np.ascontiguousarray(rhs_np[b])} for b in range(BS)]
    return pos, pooled, imgs_flat, s, in_maps


def kernel(predictions, ref_imgs):
    global _NC, LAST_RESULT
    predictions = np.asarray(predictions)
    ref_imgs = np.asarray(ref_imgs)
    pos, pooled, imgs_flat, s, in_maps = _prepare_inputs(predictions, ref_imgs)

    if _NC is None:
        _NC = _build()
    res = run_bass_kernel_spmd(_NC, in_maps, core_ids=list(range(BS)),
                               trace=TRACE)
    LAST_RESULT = res

    idxs = np.stack([np.asarray(res.results[b]["out_idx"]) for b in range(BS)])

    # [b, half, l, piece, rank] slot indices -> expand each winning fold
    # slot to its 8 source pixels: p = half*HALF + base_g + slot + (w_g/8)*t
    ci = idxs.reshape(BS, 2, L, NPIECE, 8).astype(np.int64)
    half_off = np.arange(2).reshape(1, 2, 1, 1) * HALF
    parts = []
    base = 0
    for g, nchunks in enumerate(PIECES):
        w = nchunks * CHUNK
        slots = w // SLOTPIX
        cg = np.minimum(ci[:, :, :, g, :], slots - 1)   # guard unmatched -1
        gi_g = (cg + half_off + base)[..., None] + slots * np.arange(SLOTPIX)
        parts.append(gi_g.reshape(BS, 2, L, 8 * SLOTPIX))
        base += w
    gi = np.concatenate(parts, axis=-1)
    gi2 = gi.transpose(0, 2, 1, 3).reshape(BS, L, 2 * NPIECE * 8 * SLOTPIX)

    # Exact re-rank of the captured candidates: the device's float32r values
    # are only used for *capture* (top-8 per 16384-chunk); the final ordering
    # is recomputed here in f64, which matches the reference's f32 ordering
    # because order-statistic gaps dwarf both rounding scales.
    bidx = np.arange(BS)[:, None, None]
    img_cand = imgs_flat.transpose(0, 2, 1)[bidx, gi2]      # [bs, L, 32, ch]
    s_cand = s[bidx, gi2]                                   # [bs, L, 32]
    ndv = (2.0 * np.einsum('blkc,blc->blk', img_cand.astype(np.float64),
                           pooled.astype(np.float64))
           - s_cand.astype(np.float64))

    order = np.lexsort((gi2, -ndv))          # value desc, then index asc
    top5 = np.take_along_axis(gi2, order, axis=-1)[:, :, :K]  # [bs, L, K]

    # --- remainder of the reference loss, verbatim in numpy f32 ---
    tgt_x = (top5 % IMG).astype(np.float32) / np.float32(IMG)
    tgt_y = (top5 // IMG).astype(np.float32) / np.float32(IMG)
    tgt = np.stack([tgt_x, tgt_y], axis=-1)           # [bs, L, K, 2]
    tgt_down = np.roll(tgt, shift=1, axis=1)

    d = pos[:, :, None, :] - tgt_down
    dist_down = (d * d).sum(axis=-1)                  # [bs, L, K]
    closest = np.argmin(dist_down, axis=-1)           # [bs, L]
    final_tgt = np.take_along_axis(
        tgt_down, closest[:, :, None, None], axis=2)[:, :, 0, :]

    e = pos[:, 1:] - final_tgt[:, 1:]
    loss = (e * e).sum(axis=-1)
    return np.float32(np.mean(loss))

